# revision 12
# baseline (speedup 1.0000x reference)
"""EulerFormer kernel for Trainium2 (8 NeuronCores, data-parallel over batch).

Math (per batch):
    scores = mean_l v[l, :]                          # [D]
    P      = neuralsort-softmax(scores)              # [D, D]
    vs     = v @ P.T                                 # [L, D]
    r, p   = vs[..., ::2], vs[..., 1::2]
    lam    = sqrt(r^2 + p^2 + eps)
    theta  = atan2(p, r) * delta  (== 2*atan(p/(lam+r)) * delta)
    out    = interleave(lam' * cos(theta), lam' * sin(theta)),
             lam' = lam * exp(clip(log_scale, -5, 5))

Design notes:
  - batch 16 -> 2 per core across 8 cores (SPMD).
  - P's rows are computed even-d / odd-d split (scal column reorder), so the
    einsum output per l-tile is [r(128) | p(128)] contiguous blocks -- all
    epilogue element ops run on packed APs (DVE 2x-eligible bf16).
  - DMA uses l = g*1024 + p*8 + t mapping => 8KB contiguous per partition
    per group transfer.
  - PE does all transposes (f32r, 1.5 cyc/row) then all einsums (bf16,
    1 cyc/row) back-to-back across both batches to ride the p-state ramp.
  - Scores ride the transpose-PSUM evacuation: batch0 on ACT (accum_out),
    batch1 on a custom DVE copy+accum op, balancing the two engines.
  - Epilogue: sqsum -> ACT sqrt -> fused DVE 1/(lam+r) -> Pool u=p*rec ->
    ACT atan -> fused DVE lam*cos/ lam*sin Taylor polys (|theta|<=pi*delta).
  - ACT table sets: set0 (abs+exp) for P-build, set3 (sqrt), set9 (arctan
    [+sin on the generic path]); emission order gives 3 loads total.
"""

import sys

sys.path.insert(0, "/opt/trn_rl_repo")

import numpy as np

import concourse.bacc as bacc
import concourse.mybir as mybir
import concourse.tile as tile
from concourse.bass_utils import run_bass_kernel_spmd
from concourse.tile_rust import add_dep_helper

import concourse.dve_ops as dve_ops
from concourse.dve_spec import (
    Spec, Src0, Src1, C0, C1, C2, One, Bin, AluOp, sq, _has_src1,
    lower as dve_lower,
)
from concourse.dve_uop import DveOpSpec


def _register(name, body, reference, accum=None):
    if name in dve_ops._SUB_OPCODE_FOR_NAME:
        return next(op for op in dve_ops.OPS if op.name == name)
    spec = Spec(body=body, accum=accum, reference=reference)
    row = max(dve_ops._SUB_OPCODE_FOR_NAME.values()) + 1
    assert row < 0x20, row
    op = dve_ops.DveOp(name, spec, subdim=False, uops_sha={})
    dve_ops._SUB_OPCODE_FOR_NAME[name] = row
    dve_ops.OPS.append(op)
    dve_ops.CUSTOM_DVE_SPECS[name] = spec
    for ver in ("v3", "v4"):
        try:
            compiled = DveOpSpec(
                name=name, opcode=row, uops=dve_lower(spec, ver=ver),
                rd1_en=_has_src1(spec),
            )
            op.uops_sha[ver] = compiled.sha(ver)
        except Exception:
            pass
    return op


# out = in0^2 + in1^2 + s0
SQSUM = _register(
    "TENSOR_SQSUM_ANT",
    sq(Src0) + sq(Src1) + C0,
    lambda in0, in1, c0, c1, c2: in0.astype(np.float32) ** 2
    + in1.astype(np.float32) ** 2 + c0,
)

# out ~= 1/(in0 + in1): bitwise-not exponent-flip seed + 1 Newton step
_x = Src0 + Src1
_nx = Bin(AluOp.BITWISE_NOT, _x, _x)
_y0 = _nx * C0


def _ref_recip_add(in0, in1, c0, c1, c2):
    x = in0.astype(np.float32) + in1.astype(np.float32)
    nx = (~x.view(np.int32)).view(np.float32)
    y0 = nx * c0
    return y0 * (c1 - x * y0)


RECIP_ADD = _register(
    "TENSOR_RECIP_ADD_ANT", _y0 * (C1 - _x * _y0), _ref_recip_add
)

# out = in1 * (1 + s*(c0 + s*c1)), s = in0^2   [lam * cos(c*a), deg-4]
_s = sq(Src0)
LAMCOS = _register(
    "TENSOR_LAMCOS_ANT",
    Src1 * (One + _s * (C0 + _s * C1)),
    lambda in0, in1, c0, c1, c2: in1.astype(np.float32)
    * (1.0 + (in0.astype(np.float32) ** 2)
       * (c0 + (in0.astype(np.float32) ** 2) * c1)),
)

# out = in0*in1*(c0 + s*c1)   [lam * sin(c*a), deg-3]
LAMSIN = _register(
    "TENSOR_LAMSIN_ANT",
    (Src0 * Src1) * (C0 + _s * C1),
    lambda in0, in1, c0, c1, c2: in0.astype(np.float32)
    * in1.astype(np.float32)
    * (c0 + (in0.astype(np.float32) ** 2) * c1),
)

# out = in0 (copy), accum_out = column sums (for PSUM evac + scores)
COPY_ACC = _register(
    "TENSOR_COPY_ACC_ANT",
    Src0,
    lambda in0, in1, c0, c1, c2: in0.astype(np.float32),
    accum=AluOp.ADD,
)

F32 = mybir.dt.float32
F32R = mybir.dt.float32r
BF16 = mybir.dt.bfloat16
AF = mybir.ActivationFunctionType
ALU = mybir.AluOpType
AX = mybir.AxisListType

B, L, D = 16, 4096, 256
NCORES = 8
B_PER = B // NCORES  # 2
DH = D // 2  # 128 pairs
LT = 128
NLT = L // LT  # 32 l-tiles
TAU = 1.0
EPS = 1e-6
HALF_PI = float(np.pi / 2)

G8 = 8  # l-tiles per load/transpose group
NG8 = NLT // G8  # 4
GM = 4  # l-tiles per einsum psum group
NGM = NLT // GM  # 8
CHT = 8  # l-tiles per epilogue chunk
NCH = NLT // CHT  # 4
PAIR_COLS = NLT * DH  # 4096 pair columns per batch


def _reg_consts(nc, vals):
    for val in vals:
        val = float(val)
        if (F32, val) in nc.const_aps.aps:
            continue
        t = nc.alloc_sbuf_tensor(f"const-float32-{val}", [128, 1], F32)
        nc.gpsimd.memset(t.ap(), val)
        nc.const_aps.aps[(F32, val)] = t.ap()
    nc.all_engine_barrier()


def build_program(two_delta, use_esc, use_delta_vec):
    # fast trig path: theta = two_delta * atan(u), |theta| <= |2d|*pi/2 must
    # stay small enough for the Taylor polys
    fast_trig = (not use_delta_vec) and abs(two_delta) * HALF_PI <= 0.45
    c = float(two_delta)
    # sin(c*a) = a*(c - c^3/6 a^2 + c^5/120 a^4); cos = 1 - (ca)^2/2 + ...
    sc = (c, -c**3 / 6.0, c**5 / 120.0)
    cc = (-c**2 / 2.0, c**4 / 24.0, -c**6 / 720.0)

    nc = bacc.Bacc("TRN2", target_bir_lowering=False, debug=False)
    _reg_consts(nc, [HALF_PI])

    v_d = nc.dram_tensor("v", [B_PER, L, D], F32R, kind="ExternalInput").ap()
    ident_d = nc.dram_tensor("ident", [128, 128], F32R, kind="ExternalInput").ap()
    identf_d = nc.dram_tensor("identf", [128, 128], F32, kind="ExternalInput").ap()
    scal_d = nc.dram_tensor("scalecol", [128, 2], F32, kind="ExternalInput").ap()
    if use_esc:
        esc_d = nc.dram_tensor(
            "esc_rep", [128, PAIR_COLS], F32, kind="ExternalInput"
        ).ap()
    if not fast_trig:
        d2_d = nc.dram_tensor(
            "delta2_rep", [128, PAIR_COLS], F32, kind="ExternalInput"
        ).ap()
    out_d = nc.dram_tensor("out", [B_PER, L, D], F32, kind="ExternalOutput").ap()

    rc = dve_ops.RECIP_APPROX_FAST_CONSTS

    with tile.TileContext(nc) as tc:
        with (
            tc.tile_pool(name="aux", bufs=1) as aux,
            tc.tile_pool(name="vload", bufs=3) as vload,
            tc.tile_pool(name="vt", bufs=1) as vtp,
            tc.tile_pool(name="vs", bufs=1) as vsp,
            tc.tile_pool(name="sp", bufs=1) as spp,
            tc.tile_pool(name="pb", bufs=2) as pb,
            tc.tile_pool(name="ptr", bufs=1) as ptrp,
            tc.tile_pool(name="ep", bufs=2) as ep,
            tc.tile_pool(name="outp", bufs=3) as outp,
            tc.tile_pool(name="psT", bufs=2, space="PSUM") as psT,
            tc.tile_pool(name="psMM", bufs=2, space="PSUM") as psMM,
        ):
            ident = aux.tile([128, 128], F32R, tag="ident", name="ident")
            nc.sync.dma_start(ident[:], ident_d)
            identf = aux.tile([128, 128], F32, tag="identf", name="identf")
            nc.sync.dma_start(identf[:], identf_d)
            scal = aux.tile([128, 2], F32, tag="scal", name="scal")
            nc.sync.dma_start(scal[:], scal_d)
            ones_row = aux.tile([1, 128], F32, tag="ones", name="ones")
            nc.gpsimd.memset(ones_row[:], 1.0)
            if use_esc:
                esc_t = aux.tile([128, PAIR_COLS], F32, tag="esc", name="esc")
                nc.sync.dma_start(esc_t[:], esc_d)
            if not fast_trig:
                d2_t = aux.tile([128, PAIR_COLS], F32, tag="d2", name="d2")
                nc.sync.dma_start(d2_t[:], d2_d)

            vt_t = [
                [
                    vtp.tile([128, L], BF16, tag=f"vt{bi}{ch}", name=f"vt{bi}{ch}")
                    for ch in range(2)
                ]
                for bi in range(B_PER)
            ]
            vs_t = [
                vsp.tile([128, NLT * D], BF16, tag=f"vs{bi}", name=f"vs{bi}")
                for bi in range(B_PER)
            ]
            sp_t = [
                spp.tile([128, PAIR_COLS], BF16, tag=f"sp{bi}", name=f"sp{bi}")
                for bi in range(B_PER)
            ]
            pt_t = [
                [
                    ptrp.tile([128, D], BF16, tag=f"pt{bi}{e}", name=f"pt{bi}{e}")
                    for e in range(2)
                ]
                for bi in range(B_PER)
            ]
            partials = [
                pb.tile([128, 2 * NG8], F32, tag=f"part{bi}", name=f"part{bi}", bufs=1)
                for bi in range(B_PER)
            ]

            marks = {
                "exp_last": None, "sqrt_first": None, "sqrt_last": None,
                "trig_first": None,
            }

            # ============ phase A emitters ===================================
            def emit_loads_transposes(bi):
                for g in range(NG8):
                    lv = vload.tile([128, G8 * D], F32R, tag="lv", name="lv")
                    for h in range(2):
                        hl = G8 * LT // 2
                        src = v_d[
                            bi,
                            g * G8 * LT + h * hl : g * G8 * LT + (h + 1) * hl,
                            :,
                        ].rearrange("(p t) j -> p t j", p=128)
                        nc.sync.dma_start(
                            lv[:, h * G8 * D // 2 : (h + 1) * G8 * D // 2]
                            .rearrange("p (t j) -> p t j", j=D),
                            src,
                        )
                    for ch in range(2):
                        pst = psT.tile([128, G8 * 128], F32R, tag="psT", name="psT")
                        for t in range(G8):
                            nc.tensor.transpose(
                                pst[:, t * 128 : (t + 1) * 128],
                                lv[:, t * D + ch * 128 : t * D + (ch + 1) * 128],
                                ident[:],
                            )
                        dst = vt_t[bi][ch][:, g * G8 * LT : (g + 1) * G8 * LT]
                        acc = partials[bi][:, ch * NG8 + g : ch * NG8 + g + 1]
                        nc.scalar.activation(
                            dst, pst[:], AF.Identity, accum_out=acc
                        )

            # ============ phase P emitter ====================================
            rowrep_n = [0]

            def rowrep(colpair_tile, scale, tagbase):
                ps1 = psMM.tile([128, GM * D], F32, tag="psmm", name="psmm")
                nc.tensor.transpose(ps1[0:1, 0:128], colpair_tile[:, 0:1], identf[:])
                nc.tensor.transpose(ps1[0:1, 128:256], colpair_tile[:, 1:2], identf[:])
                rowrep_n[0] += 1
                flat = pb.tile(
                    [1, D], F32, tag=f"{tagbase}f", name=f"{tagbase}f{rowrep_n[0]}"
                )
                nc.scalar.activation(flat[:], ps1[0:1, 0:D], AF.Identity, scale=scale)
                ps2 = psMM.tile([128, GM * D], F32, tag="psmm", name="psmm")
                nc.tensor.matmul(
                    ps2[:, 0:D], ones_row[:], flat[:], start=True, stop=True
                )
                rep = pb.tile(
                    [128, D], F32, tag=f"{tagbase}r", name=f"{tagbase}r{rowrep_n[0]}"
                )
                nc.scalar.activation(rep[:], ps2[:, 0:D], AF.Identity)
                return rep

            def emit_p_build(bi):
                ssum = pb.tile([128, 2], F32, tag="ssum", name=f"ssum{bi}")
                for ch in range(2):
                    nc.vector.tensor_reduce(
                        ssum[:, ch : ch + 1],
                        partials[bi][:, ch * NG8 : (ch + 1) * NG8],
                        axis=AX.X,
                        op=ALU.add,
                    )
                srow = rowrep(ssum, 1.0, f"sr{bi}")
                bsum = pb.tile([128, 2], F32, tag="bsum", name=f"bsum{bi}")
                scratch = pb.tile([128, D], F32, tag="scr", name=f"scr{bi}")
                for ch in range(2):
                    nc.vector.tensor_scalar(
                        scratch[:], srow[:], ssum[:, ch : ch + 1], None,
                        ALU.subtract,
                    )
                    nc.scalar.activation(
                        scratch[:], scratch[:], AF.Abs,
                        accum_out=bsum[:, ch : ch + 1],
                    )
                brow = rowrep(bsum, 1.0 / (L * TAU), f"br{bi}")

                expp = [None, None]
                for ch in range(2):
                    pmax = pb.tile(
                        [128, D], F32, tag=f"pmax{ch}", name=f"pmax{bi}{ch}"
                    )
                    nc.vector.scalar_tensor_tensor(
                        pmax[:], srow[:], scal[:, ch : ch + 1], brow[:],
                        ALU.mult, ALU.subtract,
                    )
                    negm = pb.tile(
                        [128, 1], F32, tag=f"negm{ch}", name=f"negm{bi}{ch}"
                    )
                    nc.vector.tensor_reduce(
                        negm[:], pmax[:], axis=AX.X, op=ALU.max, negate=True
                    )
                    rowsum = pb.tile([128, 1], F32, tag=f"rs{ch}", name=f"rs{bi}{ch}")
                    expp[ch] = pb.tile(
                        [128, D], F32, tag=f"expp{ch}", name=f"expp{bi}{ch}"
                    )
                    e_ins = nc.scalar.activation(
                        expp[ch][:], pmax[:], AF.Exp,
                        bias=negm[:], accum_out=rowsum[:],
                    )
                    marks["exp_last"] = e_ins
                    rinv = pb.tile([128, 1], F32, tag=f"ri{ch}", name=f"ri{bi}{ch}")
                    nc.vector.reciprocal_approx_fast(out=rinv[:], in_=rowsum[:])
                    nc.vector.tensor_scalar(
                        expp[ch][:], expp[ch][:], rinv[:], None, ALU.mult
                    )
                for ech in range(2):
                    ps_pt = psMM.tile([128, GM * D], F32, tag="psmm", name="psmm")
                    for dch in range(2):
                        nc.tensor.transpose(
                            ps_pt[:, dch * 128 : (dch + 1) * 128],
                            expp[dch][:, ech * 128 : (ech + 1) * 128],
                            identf[:],
                        )
                    nc.scalar.activation(
                        pt_t[bi][ech][:], ps_pt[:, 0:D], AF.Identity
                    )

            for bi in range(B_PER):
                emit_loads_transposes(bi)
                emit_p_build(bi)

            # ====== phases B+C: per-batch einsum + epilogue pipeline ====
            sqrt_marks = []
            trig_marks = []
            for bi in range(B_PER):
                vs = vs_t[bi]
                sp = sp_t[bi]
                a_tiles = []
                for g in range(NGM):
                    ps = psMM.tile([128, GM * D], F32, tag="psmm", name="psmm")
                    for t in range(GM):
                        lt = g * GM + t
                        for ch in range(2):
                            nc.tensor.matmul(
                                ps[:, t * D : (t + 1) * D],
                                vt_t[bi][ch][:, lt * 128 : (lt + 1) * 128],
                                pt_t[bi][ch][:],
                                start=(ch == 0),
                                stop=(ch == 1),
                            )
                    vc = g * GM * D
                    nc.scalar.activation(
                        vs[:, vc : vc + GM * D], ps[:, 0 : GM * D], AF.Identity
                    )
                    if g % 2 == 1:
                        cI = g // 2
                        vview = vs[:, cI * CHT * D : (cI + 1) * CHT * D].rearrange(
                            "p (t two j) -> p two t j", two=2, j=128
                        )
                        cs2 = slice(cI * CHT * DH, (cI + 1) * CHT * DH)
                        lam3 = sp[:, cs2].rearrange("p (t j) -> p t j", j=128)
                        nc.vector._custom_dve(
                            SQSUM, out=lam3, in0=vview[:, 0], in1=vview[:, 1],
                            s0=EPS,
                        )
                        s_ins = nc.scalar.activation(
                            sp[:, cs2], sp[:, cs2], AF.Sqrt
                        )
                        sqrt_marks.append(s_ins)
                        if use_esc:
                            nc.vector.tensor_tensor(
                                sp[:, cs2], sp[:, cs2], esc_t[:, cs2], ALU.mult
                            )
                        rec = ep.tile([128, CHT * DH], BF16, tag="rec", name="rec")
                        rec3 = rec[:].rearrange("p (t j) -> p t j", j=128)
                        nc.vector._custom_dve(
                            RECIP_ADD, out=rec3, in0=lam3, in1=vview[:, 0],
                            s0=rc["s0"], s1=rc["s1"],
                        )
                        u = ep.tile(
                            [128, CHT * DH], BF16, tag="u", name="u", bufs=4
                        )
                        nc.gpsimd.tensor_tensor(
                            u[:].rearrange("p (t j) -> p t j", j=128),
                            vview[:, 1], rec3, ALU.mult,
                        )
                        a_tiles.append(u)

                for cI in range(NCH):
                    u = a_tiles[cI]
                    a_t = ep.tile([128, CHT * DH], BF16, tag="at", name="at", bufs=4)
                    t_ins = nc.scalar.activation(a_t[:], u[:], AF.Arctan)
                    trig_marks.append(t_ins)
                    lam2 = sp[:, cI * CHT * DH : (cI + 1) * CHT * DH]
                    lam3 = lam2.rearrange("p (t j) -> p t j", j=128)
                    a3 = a_t[:].rearrange("p (t j) -> p t j", j=128)
                    out_t = outp.tile([128, CHT * D], F32, tag="ot", name="ot")
                    ov = out_t[:].rearrange("p (t j two) -> p two t j", two=2, j=128)
                    if fast_trig:
                        nc.vector._custom_dve(
                            LAMCOS, out=ov[:, 0], in0=a3, in1=lam3,
                            s0=cc[0], s1=cc[1],
                        )
                        nc.vector._custom_dve(
                            LAMSIN, out=ov[:, 1], in0=a3, in1=lam3,
                            s0=sc[0], s1=sc[1],
                        )
                    else:
                        cs2 = slice(cI * CHT * DH, (cI + 1) * CHT * DH)
                        th = ep.tile([128, CHT * DH], BF16, tag="th", name="th")
                        nc.vector.tensor_tensor(
                            th[:], a_t[:], d2_t[:, cs2], ALU.mult
                        )
                        ce = ep.tile([128, CHT * DH], BF16, tag="ce", name="ce")
                        nc.scalar.activation(ce[:], th[:], AF.Sin, bias=HALF_PI)
                        se = ep.tile([128, CHT * DH], BF16, tag="se", name="se")
                        nc.scalar.activation(se[:], th[:], AF.Sin, bias=0.0)
                        nc.vector.tensor_tensor(
                            ov[:, 0], lam3,
                            ce[:].rearrange("p (t j) -> p t j", j=128), ALU.mult,
                        )
                        nc.vector.tensor_tensor(
                            ov[:, 1], lam3,
                            se[:].rearrange("p (t j) -> p t j", j=128), ALU.mult,
                        )
                    for h in range(2):
                        hl = CHT * LT // 2
                        dst = out_d[
                            bi,
                            cI * CHT * LT + h * hl : cI * CHT * LT + (h + 1) * hl,
                            :,
                        ].rearrange("(p t) j -> p t j", p=128)
                        nc.sync.dma_start(
                            dst,
                            out_t[:, h * CHT * D // 2 : (h + 1) * CHT * D // 2]
                            .rearrange("p (t j) -> p t j", j=D),
                        )

            # ---- ACT table-set ordering edges ------------------------------
            if sqrt_marks and marks["exp_last"] is not None:
                add_dep_helper(
                    sqrt_marks[0].ins, marks["exp_last"].ins, sync=False,
                    reason="act-set: exps before sqrts",
                )
            # within each batch: all sqrts before first atan
            for half, tm in ((0, trig_marks[:NCH]), (1, trig_marks[NCH:])):
                sm = sqrt_marks[half * NCH : (half + 1) * NCH]
                if sm and tm:
                    add_dep_helper(
                        tm[0].ins, sm[-1].ins, sync=False,
                        reason="act-set: sqrts before trig (batch)",
                    )

    nc.compile()
    return nc


_PROGRAM_CACHE = {}


def _analyze_params(delta, log_scale):
    delta = np.asarray(delta, dtype=np.float32).reshape(-1)
    log_scale = np.asarray(log_scale, dtype=np.float32).reshape(-1)
    esc = np.exp(np.clip(log_scale, -5.0, 5.0)).astype(np.float32)
    use_esc = bool(np.any(esc != 1.0))
    use_delta_vec = bool(np.any(delta != delta[0]))
    two_delta = float(2.0 * delta[0])
    return delta, esc, use_esc, use_delta_vec, two_delta


def build_in_maps(inputs):
    v = np.ascontiguousarray(inputs["v"], dtype=np.float32)
    delta, esc, use_esc, use_delta_vec, two_delta = _analyze_params(
        inputs["delta"], inputs["log_scale"]
    )
    fast_trig = (not use_delta_vec) and abs(two_delta) * HALF_PI <= 0.45

    # scaling reordered: partition i, col ch -> d = 2i + ch (even/odd split)
    scaling = (D + 1 - 2 * (np.arange(D) + 1)).astype(np.float32) / (L * TAU)
    scal = np.ascontiguousarray(
        np.stack([scaling[0::2], scaling[1::2]], axis=1)
    ).astype(np.float32)

    shared = {
        "ident": np.eye(128, dtype=np.float32),
        "identf": np.eye(128, dtype=np.float32),
        "scalecol": scal,
    }
    # pair column layout: col = t*128 + i  (tile-major, pair i contiguous)
    if use_esc:
        shared["esc_rep"] = np.ascontiguousarray(
            np.broadcast_to(np.tile(esc, NLT)[None, :], (128, PAIR_COLS))
        ).astype(np.float32)
    if not fast_trig:
        shared["delta2_rep"] = np.ascontiguousarray(
            np.broadcast_to(
                np.tile(2.0 * delta, NLT)[None, :], (128, PAIR_COLS)
            )
        ).astype(np.float32)

    in_maps = []
    for cid in range(NCORES):
        m = dict(shared)
        m["v"] = np.ascontiguousarray(v[cid * B_PER : (cid + 1) * B_PER])
        in_maps.append(m)
    return in_maps


def kernel(v, delta, b, log_scale):
    _, _, use_esc, use_delta_vec, two_delta = _analyze_params(delta, log_scale)

    key = (use_esc, use_delta_vec, two_delta)
    if key not in _PROGRAM_CACHE:
        _PROGRAM_CACHE[key] = build_program(two_delta, use_esc, use_delta_vec)
    nc = _PROGRAM_CACHE[key]

    in_maps = build_in_maps(
        {"v": v, "delta": delta, "b": b, "log_scale": log_scale}
    )

    res = run_bass_kernel_spmd(nc, in_maps, list(range(NCORES)))
    out = np.concatenate([r["out"] for r in res.results], axis=0)
    return out.astype(np.float32)
